# revision 10
# baseline (speedup 1.0000x reference)
"""Int4Linear (dequantized int8-weight linear) for Trainium2, 8 NeuronCores.

Computes y = x @ (weight_int8 * scale[:, None]).T + bias with
  x: [4, 2048, 4096] f32, weight_int8: [16384, 4096] int32 (values in [-8, 8)),
  scale/bias: [16384] f32  ->  y: [4, 2048, 16384] f32.

Strategy: data-parallel over the 8192 token rows (1024 rows per core); every
core keeps the full weight matrix.  Host packs:
  - x shard, transposed to [128 p, KT, 1024 m] fp16 (k on partitions),
  - weights to [OT, 128 p, KT, 128 o] fp16 (exact: ints in [-8,8)),
  - scale/bias to [128, OT] f32 (per-partition columns).
Device: per o-tile, 32 k-step PSUM accumulation of matmul(lhsT=w[k,o],
rhs=x[k,m]) -> psum[o, m], then one fused Identity activation applies
scale*psum + bias and the result is DMA'd to DRAM as out.T [16384, 1024].
Host transposes each core's out.T shard back and stacks.
"""

import os
from contextlib import ExitStack

import numpy as np

import concourse.bass as bass
import concourse.tile as tile
from concourse import bacc, mybir
from concourse.bass_utils import run_bass_kernel_spmd

P = 128
N_CORES = 8
NFREE = 512  # matmul moving free dim / PSUM bank width (f32)


def build_program(din, dout, ms, n_cores=N_CORES):
    """Build + compile the per-core Bass program.

    din: contraction size, dout: global out features, ms: rows per core.
    """
    KT = din // P
    OT = dout // P
    NB = ms // NFREE
    assert din % P == 0 and dout % P == 0 and ms % NFREE == 0

    nc = bacc.Bacc(
        "TRN2", target_bir_lowering=False, debug=False, num_devices=n_cores
    )
    xt = nc.dram_tensor("xt", [P, KT, ms], mybir.dt.float16, kind="ExternalInput").ap()
    wt = nc.dram_tensor(
        "wt", [OT, P, KT, P], mybir.dt.float16, kind="ExternalInput"
    ).ap()
    sc = nc.dram_tensor("sc", [P, OT], mybir.dt.float32, kind="ExternalInput").ap()
    bs = nc.dram_tensor("bs", [P, OT], mybir.dt.float32, kind="ExternalInput").ap()
    out = nc.dram_tensor("out", [dout, ms], mybir.dt.float32, kind="ExternalOutput").ap()

    f32 = mybir.dt.float32
    f16 = mybir.dt.float16

    with tile.TileContext(nc) as tc:
        with ExitStack() as ctx:
            cpool = ctx.enter_context(tc.tile_pool(name="cpool", bufs=1))
            xpool = ctx.enter_context(tc.tile_pool(name="xpool", bufs=1))
            wpool = ctx.enter_context(tc.tile_pool(name="wpool", bufs=4))
            pspool = ctx.enter_context(tc.tile_pool(name="pspool", bufs=4, space="PSUM"))
            opool = ctx.enter_context(tc.tile_pool(name="opool", bufs=4))

            scale_sb = cpool.tile([P, OT], f32)
            nc.gpsimd.dma_start(scale_sb[:], sc[:])
            bias_sb = cpool.tile([P, OT], f32)
            nc.gpsimd.dma_start(bias_sb[:], bs[:])

            # Weights stream on the SP HWDGE queue; x + output stores on the
            # Activation HWDGE queue.  Prefetch the first weight tiles before
            # anything else so the PE starts within a few us.
            def load_w(ot, chunks=1):
                w_tile = wpool.tile(
                    [P, KT, P], f16, name=f"w_{ot}", tag="w_tile"
                )
                if chunks > 1 and KT % chunks == 0:
                    g = KT // chunks
                    for c in range(chunks):
                        nc.sync.dma_start(
                            w_tile[:, bass.ts(c, g), :], wt[ot, :, bass.ts(c, g), :]
                        )
                else:
                    nc.sync.dma_start(w_tile[:], wt[ot])
                return w_tile

            n_pre = min(2, OT)
            # Chunk the first tile's DMA so the very first matmul only waits
            # for the first k-chunk, not the whole 1 MiB tile.
            pre_w = [load_w(ot, chunks=4 if ot == 0 else 1) for ot in range(n_pre)]

            x_slab = xpool.tile([P, KT, ms], f16)
            for kt in range(KT):
                nc.scalar.dma_start(x_slab[:, kt, :], xt[:, kt, :])

            for ot in range(OT):
                w_tile = pre_w[ot] if ot < n_pre else load_w(ot)
                psums = [
                    pspool.tile([P, NFREE], f32, name=f"ps{nb}", tag=f"ps{nb}")
                    for nb in range(NB)
                ]
                for kt in range(KT):
                    lhsT = w_tile[:, kt, :]
                    for nb in range(NB):
                        nc.tensor.matmul(
                            psums[nb][:],
                            lhsT,
                            x_slab[:, kt, bass.ts(nb, NFREE)],
                            start=(kt == 0),
                            stop=(kt == KT - 1),
                        )
                for nb in range(NB):
                    o_sb = opool.tile([P, NFREE], f32, name=f"os{nb}", tag=f"os{nb}")
                    nc.scalar.activation(
                        o_sb[:],
                        psums[nb][:],
                        mybir.ActivationFunctionType.Identity,
                        bias=bias_sb[:, ot : ot + 1],
                        scale=scale_sb[:, ot : ot + 1],
                    )
                    nc.scalar.dma_start(
                        out[ot * P : (ot + 1) * P, bass.ts(nb, NFREE)], o_sb[:]
                    )
    nc.compile()
    return nc


def pack_inputs(x2d, W, scale, bias, n_cores=N_CORES):
    """Host-side shard + layout packing. Returns in_maps for run_bass_kernel_spmd."""
    M, DIN = x2d.shape
    DOUT = W.shape[0]
    MS = M // n_cores
    KT = DIN // P
    OT = DOUT // P
    f16 = np.float16

    # [OT, o, KT, p] -> [OT, p, KT, o]; ints in [-8, 8) are exact in fp16
    wt_packed = (
        W.reshape(OT, P, KT, P).transpose(0, 3, 2, 1).astype(f16, order="C")
    )
    sc_packed = np.ascontiguousarray(scale.reshape(OT, P).T).astype(np.float32)
    bs_packed = np.ascontiguousarray(bias.reshape(OT, P).T).astype(np.float32)

    in_maps = []
    for c in range(n_cores):
        xs = x2d[c * MS : (c + 1) * MS]
        xt_c = xs.reshape(MS, KT, P).transpose(2, 1, 0).astype(f16, order="C")
        in_maps.append({"xt": xt_c, "wt": wt_packed, "sc": sc_packed, "bs": bs_packed})
    return in_maps


_PROGRAM_CACHE = {}


def _get_program(din, dout, ms, n_cores):
    key = (din, dout, ms, n_cores)
    if key not in _PROGRAM_CACHE:
        _PROGRAM_CACHE[key] = build_program(din, dout, ms, n_cores)
    return _PROGRAM_CACHE[key]


def kernel(x, weight_int8, scale, bias):
    x = np.asarray(x, dtype=np.float32)
    W = np.asarray(weight_int8)
    scale = np.asarray(scale, dtype=np.float32)
    bias = np.asarray(bias, dtype=np.float32)

    B, S, DIN = x.shape
    DOUT = W.shape[0]
    M = B * S
    MS = M // N_CORES

    nc = _get_program(DIN, DOUT, MS, N_CORES)
    in_maps = pack_inputs(x.reshape(M, DIN), W, scale, bias, N_CORES)

    br = run_bass_kernel_spmd(
        nc,
        in_maps,
        list(range(N_CORES)),
        trace=bool(os.environ.get("KERNEL_TRACE")),
    )
    kernel.last_results = br

    y = np.empty((M, DOUT), dtype=np.float32)
    for c in range(N_CORES):
        y[c * MS : (c + 1) * MS] = br.results[c]["out"].T
    return y.reshape(B, S, DOUT)


kernel.last_results = None
